# revision 17
# baseline (speedup 1.0000x reference)
"""MoE layer (top-2 of 8 experts) on 8 TRN2 NeuronCores.

Strategy:
  Host: gate logits (tiny fp32 sgemm), softmax + top-2 + renormalized
      weights (the routing / sharding decision), build per-expert token
      index lists, pad to a common capacity C (multiple of 128).
  Device (expert-parallel, one launch): core e runs its expert's FFN
      over the tokens routed to it: y = relu(x@W1+b1)@W2 * w_token.
      bf16 matmuls, fp32 PSUM accumulation, weights SBUF-resident.
      Startup is HBM-bound (W1+W2+x ~21MB stream while compute starts):
      - all bulk inputs are host-packed into SBUF-tile layout so each
        dma_start moves 8KB-contiguous runs (128 descriptors, fast issue)
      - L1 of the first two token blocks is interleaved h-tile-outer,
        halving the W1 consumption rate so the 8.4MB W1 stream keeps
        ahead of the matmuls
      - W2 chunks are paced behind L1 progress via explicit deps
  Host: scatter-add the two scaled contributions per token (+ gate-
      weighted b2 correction, so b2 never rides the saturated DMA window).
"""

import numpy as np
import ml_dtypes

import concourse.mybir as mybir
import concourse.tile as tile
from concourse import bacc
from concourse.bass_utils import run_bass_kernel_spmd

P = 128
N_CORES = 8
CB = 256   # startup token block (two of these are interleaved)
CBS = 384  # steady-state token block (fewer, longer matmuls)
BF16 = mybir.dt.bfloat16
F32 = mybir.dt.float32
_bf16_np = ml_dtypes.bfloat16

_build_cache = {}

# W1 chunks (in h-tiles of 128) in consumption order; the first ones are
# small so the first matmul fires as early as possible
W1_CHUNKS = [(0, 1), (1, 1), (2, 2)]  # + [(h, 4) ...] appended per HO
HC = 4                   # h-tiles per W2 weight chunk
HG = 8                   # h-tiles per hT group tile (finer L2 deps)


def _w1_chunks(HO):
    return W1_CHUNKS + [(h, 4) for h in range(4, HO, 4)]


def _blocks(C):
    # two startup blocks of CB (interleaved L1 covers the W1 stream),
    # then CBS blocks; C % 128 == 0 leaves a 128/256 remainder at most
    starts, pos = [], 0
    while pos < C:
        if len(starts) < 2:
            cb = min(CB, C - pos)
        else:
            cb = CBS if C - pos >= CBS else C - pos
        starts.append((pos, cb))
        pos += cb
    return starts


def _build_expert(D, H, O, C):
    """Per-core expert FFN over C (padded) routed tokens.

    y[C, O] = relu(x @ W1 + b1) @ W2 * w_token[:, None]
    computed as hT = W1.T-slices @ xT (keeps H on partitions), then
    y = hT-slices.T @ W2 (tokens back on partitions). No transposes on
    device; all inputs come host-packed in SBUF-tile layout.
    """
    nc = bacc.Bacc(None, target_bir_lowering=False)
    DO, HO = D // P, H // P
    OO = O // 512
    starts = _blocks(C)
    NB = len(starts)
    NSB = min(2, NB)  # startup blocks with interleaved L1
    w1_chunks = _w1_chunks(HO)
    w1_of_hi = {}            # hi -> (chunk index, offset within chunk)
    for ci, (h0, nh) in enumerate(w1_chunks):
        for j in range(nh):
            w1_of_hi[h0 + j] = (ci, j)
    NWC = HO // HC           # number of W2 weight chunks

    # host-packed inputs: one contiguous dram tensor per SBUF tile
    w1d = [nc.dram_tensor(f"w1_{k}", [P, DO, nh * P], BF16, kind="ExternalInput")
           for k, (h0, nh) in enumerate(w1_chunks)]
    w2d = [nc.dram_tensor(f"w2_{k}", [P, HC, O], BF16, kind="ExternalInput")
           for k in range(NWC)]
    xd = [nc.dram_tensor(f"x_{b}", [P, DO, cb], BF16, kind="ExternalInput")
          for b, (n0, cb) in enumerate(starts)]
    b1 = nc.dram_tensor("b1", [P, HO], F32, kind="ExternalInput")
    wt = nc.dram_tensor("wt", [P, C // P], F32, kind="ExternalInput")
    y = nc.dram_tensor("y", [C, O], F32, kind="ExternalOutput")
    y_r = y.rearrange("(n p) o -> p n o", p=P)

    with tile.TileContext(nc) as tc:
        with (
            tc.tile_pool(name="wpool", bufs=1) as wp,
            tc.tile_pool(name="xpool", bufs=3) as xp,
            tc.tile_pool(name="hpool", bufs=2) as hp,
            tc.tile_pool(name="opool", bufs=4) as op,
            tc.tile_pool(name="hps", bufs=4, space="PSUM") as hps,
            tc.tile_pool(name="yps", bufs=3, space="PSUM") as yps,
        ):
            # -- startup-critical DMAs --
            # x0 heads the sync ring so its descriptors run ahead of the
            # W1 stream; x1/b1 head the scalar ring. W1 chunks alternate
            # across both rings so their kicks issue twice as fast and
            # the stream keeps ahead of the interleaved L1.
            w1c = [wp.tile([P, DO, nh * P], BF16, tag=f"w1_{k}", name=f"w1_{k}")
                   for k, (h0, nh) in enumerate(w1_chunks)]
            xs = {}
            for bi, ring in zip(range(NSB), (nc.sync, nc.scalar)):
                n0, cb = starts[bi]
                xt = xp.tile([P, DO, CBS], BF16, tag="x", name=f"x{bi}")
                ring.dma_start(xt[:, :, :cb], xd[bi][:])
                xs[bi] = xt[:, :, :cb]
            b1_sb = wp.tile([P, HO], F32, tag="b1")
            nc.scalar.dma_start(b1_sb[:], b1[:])
            for k in range(len(w1_chunks)):
                ring = nc.sync if k % 2 == 0 else nc.scalar
                ring.dma_start(w1c[k][:], w1d[k][:])
            wt_sb = wp.tile([P, C // P], F32, tag="wt")
            nc.scalar.dma_start(wt_sb[:], wt[:])
            w2c = [wp.tile([P, HC, O], BF16, tag=f"w2_{k}", name=f"w2_{k}")
                   for k in range(NWC)]

            # W2 chunk k streams only once startup L1 consumed W1 chunks,
            # so it never races the critical W1/x delivery
            w2_load_after = {
                max(2, 4 * k * NSB): [(w2c[k], w2d[k])]
                for k in range(NWC)
            }

            hgs = {}

            def l1_tile(bi, hi):
                n0, cb = starts[bi]
                ph = hps.tile([P, CBS], F32, tag="ph", name="ph")[:, :cb]
                ci, off = w1_of_hi[hi]
                for di in range(DO):
                    nc.tensor.matmul(
                        ph[:],
                        w1c[ci][:, di, off * P:(off + 1) * P],
                        xs[bi][:, di],
                        start=(di == 0),
                        stop=(di == DO - 1),
                    )
                return nc.scalar.activation(
                    hgs[bi][hi // HG][:, hi % HG], ph[:],
                    mybir.ActivationFunctionType.Relu,
                    bias=b1_sb[:, hi:hi + 1],
                )

            def alloc_hgs(bi):
                n0, cb = starts[bi]
                hgs[bi] = [hp.tile([P, HG, CBS], BF16, tag=f"h{g}",
                                   name=f"h{g}_{bi}")[:, :, :cb]
                           for g in range(HO // HG)]

            def drain(yp, n_idx, ot):
                o_sb = op.tile([P, 512], F32, tag="o")
                nc.vector.tensor_scalar_mul(
                    o_sb[:], yp[:], wt_sb[:, n_idx:n_idx + 1]
                )
                # y writeback on the sync ring: W1 is fully delivered by
                # the time the first drain fires
                nc.sync.dma_start(
                    y_r[:, n_idx, ot * 512:(ot + 1) * 512], o_sb[:]
                )

            def l2_block(blk):
                n0, cb = starts[blk]
                last = blk == NB - 1
                for ct in range(cb // P):
                    n_idx = n0 // P + ct
                    last_ct = last and ct == cb // P - 1
                    # hi outer / ot inner: both ot matmuls share the same
                    # stationary hT slice, halving LDWEIGHTS pressure.
                    # Final tile goes ot-outer so the first psum drains
                    # (mul + store) while the second one still matmuls.
                    if last_ct:
                        for ot in range(OO):
                            yp = yps.tile([P, 512], F32, tag="yp", name="yp")
                            for hi in range(HO):
                                nc.tensor.matmul(
                                    yp[:],
                                    hgs[blk][hi // HG][:, hi % HG, ct * P:(ct + 1) * P],
                                    w2c[hi // HC][:, hi % HC, ot * 512:(ot + 1) * 512],
                                    start=(hi == 0),
                                    stop=(hi == HO - 1),
                                )
                            drain(yp, n_idx, ot)
                    else:
                        yps_ct = [yps.tile([P, 512], F32, tag="yp", name="yp")
                                  for _ in range(OO)]
                        for hi in range(HO):
                            for ot in range(OO):
                                nc.tensor.matmul(
                                    yps_ct[ot][:],
                                    hgs[blk][hi // HG][:, hi % HG, ct * P:(ct + 1) * P],
                                    w2c[hi // HC][:, hi % HC, ot * 512:(ot + 1) * 512],
                                    start=(hi == 0),
                                    stop=(hi == HO - 1),
                                )
                        for ot in range(OO):
                            drain(yps_ct[ot], n_idx, ot)

            # interleaved L1 over the startup blocks (h-tile outer): W1
            # chunk k is needed at interleave progress k/NWC instead of
            # inside a single block's L1 — half the required stream rate
            for bi in range(NSB):
                alloc_hgs(bi)
            act_idx = 0
            for hi in range(HO):
                for bi in range(NSB):
                    act = l1_tile(bi, hi)
                    act_idx += 1
                    for w2t, w2src in w2_load_after.get(act_idx, ()):
                        dma = nc.scalar.dma_start(w2t[:], w2src[:])
                        tile.add_dep_helper(
                            dma.ins, act.ins,
                            reason="pace late load behind W1 consumption",
                        )

            # steady state: L2(0), L2(1), then L1(k)/L2(k) per block.
            # x for block k+1 is kicked one block ahead so the scalar
            # ring's in-order kicks give the DMA a full L2-block of lead.
            def prefetch_x(blk):
                n0, cb = starts[blk]
                xt = xp.tile([P, DO, CBS], BF16, tag="x", name=f"x{blk}")
                nc.scalar.dma_start(xt[:, :, :cb], xd[blk][:])
                xs[blk] = xt[:, :, :cb]

            for blk in range(NB):
                if blk + 1 < NB and blk + 1 >= NSB:
                    prefetch_x(blk + 1)
                if blk >= NSB:
                    alloc_hgs(blk)
                    for hi in range(HO):
                        l1_tile(blk, hi)
                l2_block(blk)
    nc.finalize()
    return nc


def kernel(x, W1, b1, W2, b2, gate_w, gate_b):
    x = np.ascontiguousarray(x, dtype=np.float32)
    W1 = np.asarray(W1, dtype=np.float32)
    b1 = np.asarray(b1, dtype=np.float32)
    W2 = np.asarray(W2, dtype=np.float32)
    b2 = np.asarray(b2, dtype=np.float32)
    gate_w = np.ascontiguousarray(gate_w, dtype=np.float32)
    gate_b = np.asarray(gate_b, dtype=np.float32)

    B, D = x.shape
    E, _, H = W1.shape
    O = W2.shape[2]
    DO, HO = D // P, H // P
    assert E == N_CORES and D % P == 0 and H % (P * HC) == 0 and O % 512 == 0
    core_ids = list(range(N_CORES))

    # ---- Host: gate logits + top-2 routing (the sharding decision) ----
    logits = x @ gate_w + gate_b
    lg = logits.astype(np.float64)
    lg -= lg.max(axis=1, keepdims=True)
    probs = np.exp(lg)
    probs /= probs.sum(axis=1, keepdims=True)
    order = np.argsort(-probs, axis=1, kind="stable")[:, :2]
    p_top = np.take_along_axis(probs, order, axis=1)
    w_top = (p_top / p_top.sum(axis=1, keepdims=True)).astype(np.float32)  # [B, 2]

    idx_e, wt_e = [], []
    for e in range(E):
        m0 = order[:, 0] == e
        m1 = order[:, 1] == e
        sel = m0 | m1
        idx = np.nonzero(sel)[0]
        w = np.where(m0[sel], w_top[sel, 0], w_top[sel, 1]).astype(np.float32)
        idx_e.append(idx)
        wt_e.append(w)
    max_count = max(len(i) for i in idx_e)
    C = max(CB, ((max_count + P - 1) // P) * P)
    starts = _blocks(C)

    # ---- Device: expert FFN (expert-parallel, one launch) ----
    key = ("expert", D, H, O, C)
    if key not in _build_cache:
        _build_cache[key] = _build_expert(D, H, O, C)
    nc_exp = _build_cache[key]

    w1_chunks = _w1_chunks(HO)
    NWC = HO // HC
    in_maps = []
    for e in range(E):
        n_e = len(idx_e[e])
        xT_pad = np.zeros((C, D), dtype=_bf16_np)
        xT_pad[:n_e] = x[idx_e[e]].astype(_bf16_np)
        x_r = xT_pad.reshape(C, DO, P)
        wt_pad = np.zeros(C, dtype=np.float32)
        wt_pad[:n_e] = wt_e[e]
        w1_r = W1[e].astype(_bf16_np).reshape(DO, P, H)
        w2_r = W2[e].astype(_bf16_np).reshape(HO, P, O)
        im = {
            "b1": np.ascontiguousarray(b1[e].reshape(HO, P).T),
            "wt": np.ascontiguousarray(wt_pad.reshape(C // P, P).T),
        }
        for k, (h0, nh) in enumerate(w1_chunks):
            im[f"w1_{k}"] = np.ascontiguousarray(
                w1_r[:, :, h0 * P:(h0 + nh) * P].transpose(1, 0, 2))
        for k in range(NWC):
            im[f"w2_{k}"] = np.ascontiguousarray(
                w2_r[k * HC:(k + 1) * HC].transpose(1, 0, 2))
        for b, (n0, cb) in enumerate(starts):
            im[f"x_{b}"] = np.ascontiguousarray(
                x_r[n0:n0 + cb].transpose(2, 1, 0))
        in_maps.append(im)
    res = run_bass_kernel_spmd(nc_exp, in_maps, core_ids=core_ids)

    # ---- Host: un-permute and combine the two expert contributions ----
    out = np.zeros((B, O), dtype=np.float32)
    for e in range(E):
        n_e = len(idx_e[e])
        if n_e:
            out[idx_e[e]] += res.results[e]["y"][:n_e]
    if np.any(b2):
        # b2 is applied host-side: out += sum_k w_k * b2[expert_k]
        out += w_top[:, 0:1] * b2[order[:, 0]] + w_top[:, 1:2] * b2[order[:, 1]]
    return out


# revision 19
# speedup vs baseline: 1.0244x; 1.0244x over previous
"""MoE layer (top-2 of 8 experts) on 8 TRN2 NeuronCores.

Strategy:
  Host: gate logits (tiny fp32 sgemm), softmax + top-2 + renormalized
      weights (the routing / sharding decision), build per-expert token
      index lists, pad to a common capacity C (multiple of 128).
  Device (expert-parallel, one launch): core e runs its expert's FFN
      over the tokens routed to it: y = relu(x@W1+b1)@W2 * w_token.
      bf16 matmuls, fp32 PSUM accumulation, weights SBUF-resident.
      Startup is HBM-bound (W1+W2+x ~21MB stream while compute starts):
      - all bulk inputs are host-packed into SBUF-tile layout so each
        dma_start moves 8KB-contiguous runs (128 descriptors, fast issue)
      - L1 of the first two token blocks is interleaved h-tile-outer,
        halving the W1 consumption rate so the 8.4MB W1 stream keeps
        ahead of the matmuls
      - W2 chunks are paced behind L1 progress via explicit deps
  Host: scatter-add the two scaled contributions per token (+ gate-
      weighted b2 correction, so b2 never rides the saturated DMA window).
"""

import numpy as np
import ml_dtypes

import concourse.mybir as mybir
import concourse.tile as tile
from concourse import bacc
from concourse.bass_utils import run_bass_kernel_spmd

P = 128
N_CORES = 8
CB = 256   # startup token block (two of these are interleaved)
CBS = 384  # steady-state token block (fewer, longer matmuls)
BF16 = mybir.dt.bfloat16
F32 = mybir.dt.float32
_bf16_np = ml_dtypes.bfloat16

_build_cache = {}

# W1 chunks (in h-tiles of 128) in consumption order; the first ones are
# small so the first matmul fires as early as possible
W1_CHUNKS = [(0, 1), (1, 1), (2, 2)]  # + [(h, 4) ...] appended per HO
HC = 4                   # h-tiles per W2 weight chunk
HG = 8                   # h-tiles per hT group tile (finer L2 deps)


def _w1_chunks(HO):
    return W1_CHUNKS + [(h, 4) for h in range(4, HO, 4)]


def _blocks(C):
    # one small startup block of CB (its x rides ahead of the W1 stream;
    # blocks 0+1 run L1 interleaved to cover it), then CBS blocks;
    # C % 128 == 0 leaves a 128/256 remainder at most
    starts, pos = [], 0
    while pos < C:
        if not starts:
            cb = min(CB, C - pos)
        else:
            cb = CBS if C - pos >= CBS else C - pos
        starts.append((pos, cb))
        pos += cb
    return starts


def _build_expert(D, H, O, C):
    """Per-core expert FFN over C (padded) routed tokens.

    y[C, O] = relu(x @ W1 + b1) @ W2 * w_token[:, None]
    computed as hT = W1.T-slices @ xT (keeps H on partitions), then
    y = hT-slices.T @ W2 (tokens back on partitions). No transposes on
    device; all inputs come host-packed in SBUF-tile layout.
    """
    nc = bacc.Bacc(None, target_bir_lowering=False)
    DO, HO = D // P, H // P
    OO = O // 512
    starts = _blocks(C)
    NB = len(starts)
    NSB = min(2, NB)  # startup blocks with interleaved L1
    w1_chunks = _w1_chunks(HO)
    w1_of_hi = {}            # hi -> (chunk index, offset within chunk)
    for ci, (h0, nh) in enumerate(w1_chunks):
        for j in range(nh):
            w1_of_hi[h0 + j] = (ci, j)
    NWC = HO // HC           # number of W2 weight chunks

    # host-packed inputs: one contiguous dram tensor per SBUF tile
    w1d = [nc.dram_tensor(f"w1_{k}", [P, DO, nh * P], BF16, kind="ExternalInput")
           for k, (h0, nh) in enumerate(w1_chunks)]
    w2d = [nc.dram_tensor(f"w2_{k}", [P, HC, O], BF16, kind="ExternalInput")
           for k in range(NWC)]
    xd = [nc.dram_tensor(f"x_{b}", [P, DO, cb], BF16, kind="ExternalInput")
          for b, (n0, cb) in enumerate(starts)]
    b1 = nc.dram_tensor("b1", [P, HO], F32, kind="ExternalInput")
    wt = nc.dram_tensor("wt", [P, C // P], F32, kind="ExternalInput")
    y = nc.dram_tensor("y", [C, O], F32, kind="ExternalOutput")
    y_r = y.rearrange("(n p) o -> p n o", p=P)

    with tile.TileContext(nc) as tc:
        with (
            tc.tile_pool(name="wpool", bufs=1) as wp,
            tc.tile_pool(name="xpool", bufs=3) as xp,
            tc.tile_pool(name="hpool", bufs=2) as hp,
            tc.tile_pool(name="opool", bufs=4) as op,
            tc.tile_pool(name="hps", bufs=4, space="PSUM") as hps,
            tc.tile_pool(name="yps", bufs=3, space="PSUM") as yps,
        ):
            # -- startup-critical DMAs --
            # x0 heads the sync ring so its descriptors run ahead of the
            # W1 stream; x1/b1 head the scalar ring. W1 chunks alternate
            # across both rings so their kicks issue twice as fast and
            # the stream keeps ahead of the interleaved L1.
            w1c = [wp.tile([P, DO, nh * P], BF16, tag=f"w1_{k}", name=f"w1_{k}")
                   for k, (h0, nh) in enumerate(w1_chunks)]
            xs = {}
            for bi, ring in zip(range(NSB), (nc.sync, nc.scalar)):
                n0, cb = starts[bi]
                xt = xp.tile([P, DO, CBS], BF16, tag="x", name=f"x{bi}")
                ring.dma_start(xt[:, :, :cb], xd[bi][:])
                xs[bi] = xt[:, :, :cb]
            b1_sb = wp.tile([P, HO], F32, tag="b1")
            nc.scalar.dma_start(b1_sb[:], b1[:])
            for k in range(len(w1_chunks)):
                nc.sync.dma_start(w1c[k][:], w1d[k][:])
            wt_sb = wp.tile([P, C // P], F32, tag="wt")
            nc.scalar.dma_start(wt_sb[:], wt[:])
            w2c = [wp.tile([P, HC, O], BF16, tag=f"w2_{k}", name=f"w2_{k}")
                   for k in range(NWC)]

            # W2 chunk k streams only once startup L1 consumed W1 chunks,
            # so it never races the critical W1/x delivery
            w2_load_after = {
                max(2, 4 * k * NSB): [(w2c[k], w2d[k])]
                for k in range(NWC)
            }

            hgs = {}

            def l1_tile(bi, hi):
                n0, cb = starts[bi]
                ph = hps.tile([P, CBS], F32, tag="ph", name="ph")[:, :cb]
                ci, off = w1_of_hi[hi]
                for di in range(DO):
                    nc.tensor.matmul(
                        ph[:],
                        w1c[ci][:, di, off * P:(off + 1) * P],
                        xs[bi][:, di],
                        start=(di == 0),
                        stop=(di == DO - 1),
                    )
                return nc.scalar.activation(
                    hgs[bi][hi // HG][:, hi % HG], ph[:],
                    mybir.ActivationFunctionType.Relu,
                    bias=b1_sb[:, hi:hi + 1],
                )

            def alloc_hgs(bi):
                n0, cb = starts[bi]
                hgs[bi] = [hp.tile([P, HG, CBS], BF16, tag=f"h{g}",
                                   name=f"h{g}_{bi}")[:, :, :cb]
                           for g in range(HO // HG)]

            def drain(yp, n_idx, ot):
                o_sb = op.tile([P, 512], F32, tag="o")
                nc.vector.tensor_scalar_mul(
                    o_sb[:], yp[:], wt_sb[:, n_idx:n_idx + 1]
                )
                # y writeback on the sync ring: W1 is fully delivered by
                # the time the first drain fires
                nc.sync.dma_start(
                    y_r[:, n_idx, ot * 512:(ot + 1) * 512], o_sb[:]
                )

            def l2_block(blk):
                n0, cb = starts[blk]
                last = blk == NB - 1
                for ct in range(cb // P):
                    n_idx = n0 // P + ct
                    last_ct = last and ct == cb // P - 1
                    # hi outer / ot inner: both ot matmuls share the same
                    # stationary hT slice, halving LDWEIGHTS pressure.
                    # Final tile goes ot-outer so the first psum drains
                    # (mul + store) while the second one still matmuls.
                    if last_ct:
                        for ot in range(OO):
                            yp = yps.tile([P, 512], F32, tag="yp", name="yp")
                            for hi in range(HO):
                                nc.tensor.matmul(
                                    yp[:],
                                    hgs[blk][hi // HG][:, hi % HG, ct * P:(ct + 1) * P],
                                    w2c[hi // HC][:, hi % HC, ot * 512:(ot + 1) * 512],
                                    start=(hi == 0),
                                    stop=(hi == HO - 1),
                                )
                            drain(yp, n_idx, ot)
                    else:
                        yps_ct = [yps.tile([P, 512], F32, tag="yp", name="yp")
                                  for _ in range(OO)]
                        for hi in range(HO):
                            for ot in range(OO):
                                nc.tensor.matmul(
                                    yps_ct[ot][:],
                                    hgs[blk][hi // HG][:, hi % HG, ct * P:(ct + 1) * P],
                                    w2c[hi // HC][:, hi % HC, ot * 512:(ot + 1) * 512],
                                    start=(hi == 0),
                                    stop=(hi == HO - 1),
                                )
                        for ot in range(OO):
                            drain(yps_ct[ot], n_idx, ot)

            # interleaved L1 over the startup blocks (h-tile outer): W1
            # chunk k is needed at interleave progress k/NWC instead of
            # inside a single block's L1 — half the required stream rate
            for bi in range(NSB):
                alloc_hgs(bi)
            act_idx = 0
            for hi in range(HO):
                for bi in range(NSB):
                    act = l1_tile(bi, hi)
                    act_idx += 1
                    for w2t, w2src in w2_load_after.get(act_idx, ()):
                        dma = nc.scalar.dma_start(w2t[:], w2src[:])
                        tile.add_dep_helper(
                            dma.ins, act.ins,
                            reason="pace late load behind W1 consumption",
                        )

            # steady state: L2(0), L2(1), then L1(k)/L2(k) per block.
            # x for block k+1 is kicked one block ahead so the scalar
            # ring's in-order kicks give the DMA a full L2-block of lead.
            def prefetch_x(blk):
                n0, cb = starts[blk]
                xt = xp.tile([P, DO, CBS], BF16, tag="x", name=f"x{blk}")
                nc.scalar.dma_start(xt[:, :, :cb], xd[blk][:])
                xs[blk] = xt[:, :, :cb]

            for blk in range(NB):
                if blk + 1 < NB and blk + 1 >= NSB:
                    prefetch_x(blk + 1)
                if blk >= NSB:
                    alloc_hgs(blk)
                    for hi in range(HO):
                        l1_tile(blk, hi)
                l2_block(blk)
    nc.finalize()
    return nc


def kernel(x, W1, b1, W2, b2, gate_w, gate_b):
    x = np.ascontiguousarray(x, dtype=np.float32)
    W1 = np.asarray(W1, dtype=np.float32)
    b1 = np.asarray(b1, dtype=np.float32)
    W2 = np.asarray(W2, dtype=np.float32)
    b2 = np.asarray(b2, dtype=np.float32)
    gate_w = np.ascontiguousarray(gate_w, dtype=np.float32)
    gate_b = np.asarray(gate_b, dtype=np.float32)

    B, D = x.shape
    E, _, H = W1.shape
    O = W2.shape[2]
    DO, HO = D // P, H // P
    assert E == N_CORES and D % P == 0 and H % (P * HC) == 0 and O % 512 == 0
    core_ids = list(range(N_CORES))

    # ---- Host: gate logits + top-2 routing (the sharding decision) ----
    logits = x @ gate_w + gate_b
    lg = logits.astype(np.float64)
    lg -= lg.max(axis=1, keepdims=True)
    probs = np.exp(lg)
    probs /= probs.sum(axis=1, keepdims=True)
    order = np.argsort(-probs, axis=1, kind="stable")[:, :2]
    p_top = np.take_along_axis(probs, order, axis=1)
    w_top = (p_top / p_top.sum(axis=1, keepdims=True)).astype(np.float32)  # [B, 2]

    idx_e, wt_e = [], []
    for e in range(E):
        m0 = order[:, 0] == e
        m1 = order[:, 1] == e
        sel = m0 | m1
        idx = np.nonzero(sel)[0]
        w = np.where(m0[sel], w_top[sel, 0], w_top[sel, 1]).astype(np.float32)
        idx_e.append(idx)
        wt_e.append(w)
    max_count = max(len(i) for i in idx_e)
    C = max(CB, ((max_count + P - 1) // P) * P)
    starts = _blocks(C)

    # ---- Device: expert FFN (expert-parallel, one launch) ----
    key = ("expert", D, H, O, C)
    if key not in _build_cache:
        _build_cache[key] = _build_expert(D, H, O, C)
    nc_exp = _build_cache[key]

    w1_chunks = _w1_chunks(HO)
    NWC = HO // HC
    in_maps = []
    for e in range(E):
        n_e = len(idx_e[e])
        xT_pad = np.zeros((C, D), dtype=_bf16_np)
        xT_pad[:n_e] = x[idx_e[e]].astype(_bf16_np)
        x_r = xT_pad.reshape(C, DO, P)
        wt_pad = np.zeros(C, dtype=np.float32)
        wt_pad[:n_e] = wt_e[e]
        w1_r = W1[e].astype(_bf16_np).reshape(DO, P, H)
        w2_r = W2[e].astype(_bf16_np).reshape(HO, P, O)
        im = {
            "b1": np.ascontiguousarray(b1[e].reshape(HO, P).T),
            "wt": np.ascontiguousarray(wt_pad.reshape(C // P, P).T),
        }
        for k, (h0, nh) in enumerate(w1_chunks):
            im[f"w1_{k}"] = np.ascontiguousarray(
                w1_r[:, :, h0 * P:(h0 + nh) * P].transpose(1, 0, 2))
        for k in range(NWC):
            im[f"w2_{k}"] = np.ascontiguousarray(
                w2_r[k * HC:(k + 1) * HC].transpose(1, 0, 2))
        for b, (n0, cb) in enumerate(starts):
            im[f"x_{b}"] = np.ascontiguousarray(
                x_r[n0:n0 + cb].transpose(2, 1, 0))
        in_maps.append(im)
    res = run_bass_kernel_spmd(nc_exp, in_maps, core_ids=core_ids)

    # ---- Host: un-permute and combine the two expert contributions ----
    out = np.zeros((B, O), dtype=np.float32)
    for e in range(E):
        n_e = len(idx_e[e])
        if n_e:
            out[idx_e[e]] += res.results[e]["y"][:n_e]
    if np.any(b2):
        # b2 is applied host-side: out += sum_k w_k * b2[expert_k]
        out += w_top[:, 0:1] * b2[order[:, 0]] + w_top[:, 1:2] * b2[order[:, 1]]
    return out


# revision 20
# speedup vs baseline: 1.0248x; 1.0004x over previous
"""MoE layer (top-2 of 8 experts) on 8 TRN2 NeuronCores.

Strategy:
  Host: gate logits (tiny fp32 sgemm), softmax + top-2 + renormalized
      weights (the routing / sharding decision), build per-expert token
      index lists, pad to a common capacity C (multiple of 128).
  Device (expert-parallel, one launch): core e runs its expert's FFN
      over the tokens routed to it: y = relu(x@W1+b1)@W2 * w_token.
      bf16 matmuls, fp32 PSUM accumulation, weights SBUF-resident.
      Startup is HBM-bound (W1+W2+x ~21MB stream while compute starts):
      - all bulk inputs are host-packed into SBUF-tile layout so each
        dma_start moves 8KB-contiguous runs (128 descriptors, fast issue)
      - L1 of the first two token blocks is interleaved h-tile-outer,
        halving the W1 consumption rate so the 8.4MB W1 stream keeps
        ahead of the matmuls
      - W2 chunks are paced behind L1 progress via explicit deps
  Host: scatter-add the two scaled contributions per token (+ gate-
      weighted b2 correction, so b2 never rides the saturated DMA window).
"""

import numpy as np
import ml_dtypes

import concourse.mybir as mybir
import concourse.tile as tile
from concourse import bacc
from concourse.bass_utils import run_bass_kernel_spmd

P = 128
N_CORES = 8
CB = 256   # startup token block (two of these are interleaved)
CBS = 384  # steady-state token block (fewer, longer matmuls)
BF16 = mybir.dt.bfloat16
F32 = mybir.dt.float32
_bf16_np = ml_dtypes.bfloat16

_build_cache = {}

# W1 chunks (in h-tiles of 128) in consumption order; the first ones are
# small so the first matmul fires as early as possible
W1_CHUNKS = [(0, 1), (1, 1), (2, 2)]  # + [(h, 4) ...] appended per HO
HC = 4                   # h-tiles per W2 weight chunk
HG = 8                   # h-tiles per hT group tile (finer L2 deps)


def _w1_chunks(HO):
    return W1_CHUNKS + [(h, 4) for h in range(4, HO, 4)]


def _blocks(C):
    # one small startup block of CB (its x rides ahead of the W1 stream;
    # blocks 0+1 run L1 interleaved to cover it), then CBS blocks;
    # C % 128 == 0 leaves a 128/256 remainder at most
    starts, pos = [], 0
    while pos < C:
        if not starts:
            cb = min(CB, C - pos)
        else:
            cb = CBS if C - pos >= CBS else C - pos
        starts.append((pos, cb))
        pos += cb
    return starts


def _build_expert(D, H, O, C):
    """Per-core expert FFN over C (padded) routed tokens.

    y[C, O] = relu(x @ W1 + b1) @ W2 * w_token[:, None]
    computed as hT = W1.T-slices @ xT (keeps H on partitions), then
    y = hT-slices.T @ W2 (tokens back on partitions). No transposes on
    device; all inputs come host-packed in SBUF-tile layout.
    """
    nc = bacc.Bacc(None, target_bir_lowering=False)
    DO, HO = D // P, H // P
    OO = O // 512
    starts = _blocks(C)
    NB = len(starts)
    NSB = min(2, NB)  # startup blocks with interleaved L1
    w1_chunks = _w1_chunks(HO)
    w1_of_hi = {}            # hi -> (chunk index, offset within chunk)
    for ci, (h0, nh) in enumerate(w1_chunks):
        for j in range(nh):
            w1_of_hi[h0 + j] = (ci, j)
    NWC = HO // HC           # number of W2 weight chunks

    # host-packed inputs: one contiguous dram tensor per SBUF tile
    w1d = [nc.dram_tensor(f"w1_{k}", [P, DO, nh * P], BF16, kind="ExternalInput")
           for k, (h0, nh) in enumerate(w1_chunks)]
    w2d = [nc.dram_tensor(f"w2_{k}", [P, HC, O], BF16, kind="ExternalInput")
           for k in range(NWC)]
    xd = [nc.dram_tensor(f"x_{b}", [P, DO, cb], BF16, kind="ExternalInput")
          for b, (n0, cb) in enumerate(starts)]
    b1 = nc.dram_tensor("b1", [P, HO], F32, kind="ExternalInput")
    wt = nc.dram_tensor("wt", [P, C // P], F32, kind="ExternalInput")
    y = nc.dram_tensor("y", [C, O], F32, kind="ExternalOutput")
    y_r = y.rearrange("(n p) o -> p n o", p=P)

    with tile.TileContext(nc) as tc:
        with (
            tc.tile_pool(name="wpool", bufs=1) as wp,
            tc.tile_pool(name="xpool", bufs=3) as xp,
            tc.tile_pool(name="hpool", bufs=2) as hp,
            tc.tile_pool(name="opool", bufs=4) as op,
            tc.tile_pool(name="hps", bufs=4, space="PSUM") as hps,
            tc.tile_pool(name="yps", bufs=3, space="PSUM") as yps,
        ):
            # -- startup-critical DMAs --
            # x0 heads the sync ring so its descriptors run ahead of the
            # W1 stream; x1/b1 head the scalar ring. W1 chunks alternate
            # across both rings so their kicks issue twice as fast and
            # the stream keeps ahead of the interleaved L1.
            w1c = [wp.tile([P, DO, nh * P], BF16, tag=f"w1_{k}", name=f"w1_{k}")
                   for k, (h0, nh) in enumerate(w1_chunks)]
            xs = {}
            for bi in range(NSB):
                n0, cb = starts[bi]
                xt = xp.tile([P, DO, CBS], BF16, tag="x", name=f"x{bi}")
                xs[bi] = xt[:, :, :cb]
            nc.sync.dma_start(xs[0][:], xd[0][:])
            b1_sb = wp.tile([P, HO], F32, tag="b1")
            if NSB > 1:
                # block-1 x in di-halves: only the first half competes with
                # x0/W1 in the critical first-matmul window
                nc.scalar.dma_start(xs[1][:, :DO // 2], xd[1][:, :DO // 2])
                nc.scalar.dma_start(b1_sb[:], b1[:])
                nc.scalar.dma_start(xs[1][:, DO // 2:], xd[1][:, DO // 2:])
            else:
                nc.scalar.dma_start(b1_sb[:], b1[:])
            for k in range(len(w1_chunks)):
                nc.sync.dma_start(w1c[k][:], w1d[k][:])
            wt_sb = wp.tile([P, C // P], F32, tag="wt")
            nc.scalar.dma_start(wt_sb[:], wt[:])
            w2c = [wp.tile([P, HC, O], BF16, tag=f"w2_{k}", name=f"w2_{k}")
                   for k in range(NWC)]

            # W2 chunk k streams only once startup L1 consumed W1 chunks,
            # so it never races the critical W1/x delivery
            w2_load_after = {
                max(2, 4 * k * NSB): [(w2c[k], w2d[k])]
                for k in range(NWC)
            }

            hgs = {}

            def l1_tile(bi, hi):
                n0, cb = starts[bi]
                ph = hps.tile([P, CBS], F32, tag="ph", name="ph")[:, :cb]
                ci, off = w1_of_hi[hi]
                for di in range(DO):
                    nc.tensor.matmul(
                        ph[:],
                        w1c[ci][:, di, off * P:(off + 1) * P],
                        xs[bi][:, di],
                        start=(di == 0),
                        stop=(di == DO - 1),
                    )
                return nc.scalar.activation(
                    hgs[bi][hi // HG][:, hi % HG], ph[:],
                    mybir.ActivationFunctionType.Relu,
                    bias=b1_sb[:, hi:hi + 1],
                )

            def alloc_hgs(bi):
                n0, cb = starts[bi]
                hgs[bi] = [hp.tile([P, HG, CBS], BF16, tag=f"h{g}",
                                   name=f"h{g}_{bi}")[:, :, :cb]
                           for g in range(HO // HG)]

            def drain(yp, n_idx, ot):
                o_sb = op.tile([P, 512], F32, tag="o")
                nc.vector.tensor_scalar_mul(
                    o_sb[:], yp[:], wt_sb[:, n_idx:n_idx + 1]
                )
                # y writeback on the sync ring: W1 is fully delivered by
                # the time the first drain fires
                nc.sync.dma_start(
                    y_r[:, n_idx, ot * 512:(ot + 1) * 512], o_sb[:]
                )

            def l2_block(blk):
                n0, cb = starts[blk]
                last = blk == NB - 1
                for ct in range(cb // P):
                    n_idx = n0 // P + ct
                    last_ct = last and ct == cb // P - 1
                    # hi outer / ot inner: both ot matmuls share the same
                    # stationary hT slice, halving LDWEIGHTS pressure.
                    # Final tile goes ot-outer so the first psum drains
                    # (mul + store) while the second one still matmuls.
                    if last_ct:
                        for ot in range(OO):
                            yp = yps.tile([P, 512], F32, tag="yp", name="yp")
                            for hi in range(HO):
                                nc.tensor.matmul(
                                    yp[:],
                                    hgs[blk][hi // HG][:, hi % HG, ct * P:(ct + 1) * P],
                                    w2c[hi // HC][:, hi % HC, ot * 512:(ot + 1) * 512],
                                    start=(hi == 0),
                                    stop=(hi == HO - 1),
                                )
                            drain(yp, n_idx, ot)
                    else:
                        yps_ct = [yps.tile([P, 512], F32, tag="yp", name="yp")
                                  for _ in range(OO)]
                        for hi in range(HO):
                            for ot in range(OO):
                                nc.tensor.matmul(
                                    yps_ct[ot][:],
                                    hgs[blk][hi // HG][:, hi % HG, ct * P:(ct + 1) * P],
                                    w2c[hi // HC][:, hi % HC, ot * 512:(ot + 1) * 512],
                                    start=(hi == 0),
                                    stop=(hi == HO - 1),
                                )
                        for ot in range(OO):
                            drain(yps_ct[ot], n_idx, ot)

            # interleaved L1 over the startup blocks (h-tile outer): W1
            # chunk k is needed at interleave progress k/NWC instead of
            # inside a single block's L1 — half the required stream rate
            for bi in range(NSB):
                alloc_hgs(bi)
            act_idx = 0
            for hi in range(HO):
                for bi in range(NSB):
                    act = l1_tile(bi, hi)
                    act_idx += 1
                    for w2t, w2src in w2_load_after.get(act_idx, ()):
                        dma = nc.scalar.dma_start(w2t[:], w2src[:])
                        tile.add_dep_helper(
                            dma.ins, act.ins,
                            reason="pace late load behind W1 consumption",
                        )

            # steady state: L2(0), L2(1), then L1(k)/L2(k) per block.
            # x for block k+1 is kicked one block ahead so the scalar
            # ring's in-order kicks give the DMA a full L2-block of lead.
            def prefetch_x(blk):
                n0, cb = starts[blk]
                xt = xp.tile([P, DO, CBS], BF16, tag="x", name=f"x{blk}")
                nc.scalar.dma_start(xt[:, :, :cb], xd[blk][:])
                xs[blk] = xt[:, :, :cb]

            for blk in range(NB):
                if blk + 1 < NB and blk + 1 >= NSB:
                    prefetch_x(blk + 1)
                if blk >= NSB:
                    alloc_hgs(blk)
                    for hi in range(HO):
                        l1_tile(blk, hi)
                l2_block(blk)
    nc.finalize()
    return nc


def kernel(x, W1, b1, W2, b2, gate_w, gate_b):
    x = np.ascontiguousarray(x, dtype=np.float32)
    W1 = np.asarray(W1, dtype=np.float32)
    b1 = np.asarray(b1, dtype=np.float32)
    W2 = np.asarray(W2, dtype=np.float32)
    b2 = np.asarray(b2, dtype=np.float32)
    gate_w = np.ascontiguousarray(gate_w, dtype=np.float32)
    gate_b = np.asarray(gate_b, dtype=np.float32)

    B, D = x.shape
    E, _, H = W1.shape
    O = W2.shape[2]
    DO, HO = D // P, H // P
    assert E == N_CORES and D % P == 0 and H % (P * HC) == 0 and O % 512 == 0
    core_ids = list(range(N_CORES))

    # ---- Host: gate logits + top-2 routing (the sharding decision) ----
    logits = x @ gate_w + gate_b
    lg = logits.astype(np.float64)
    lg -= lg.max(axis=1, keepdims=True)
    probs = np.exp(lg)
    probs /= probs.sum(axis=1, keepdims=True)
    order = np.argsort(-probs, axis=1, kind="stable")[:, :2]
    p_top = np.take_along_axis(probs, order, axis=1)
    w_top = (p_top / p_top.sum(axis=1, keepdims=True)).astype(np.float32)  # [B, 2]

    idx_e, wt_e = [], []
    for e in range(E):
        m0 = order[:, 0] == e
        m1 = order[:, 1] == e
        sel = m0 | m1
        idx = np.nonzero(sel)[0]
        w = np.where(m0[sel], w_top[sel, 0], w_top[sel, 1]).astype(np.float32)
        idx_e.append(idx)
        wt_e.append(w)
    max_count = max(len(i) for i in idx_e)
    C = max(CB, ((max_count + P - 1) // P) * P)
    starts = _blocks(C)

    # ---- Device: expert FFN (expert-parallel, one launch) ----
    key = ("expert", D, H, O, C)
    if key not in _build_cache:
        _build_cache[key] = _build_expert(D, H, O, C)
    nc_exp = _build_cache[key]

    w1_chunks = _w1_chunks(HO)
    NWC = HO // HC
    in_maps = []
    for e in range(E):
        n_e = len(idx_e[e])
        xT_pad = np.zeros((C, D), dtype=_bf16_np)
        xT_pad[:n_e] = x[idx_e[e]].astype(_bf16_np)
        x_r = xT_pad.reshape(C, DO, P)
        wt_pad = np.zeros(C, dtype=np.float32)
        wt_pad[:n_e] = wt_e[e]
        w1_r = W1[e].astype(_bf16_np).reshape(DO, P, H)
        w2_r = W2[e].astype(_bf16_np).reshape(HO, P, O)
        im = {
            "b1": np.ascontiguousarray(b1[e].reshape(HO, P).T),
            "wt": np.ascontiguousarray(wt_pad.reshape(C // P, P).T),
        }
        for k, (h0, nh) in enumerate(w1_chunks):
            im[f"w1_{k}"] = np.ascontiguousarray(
                w1_r[:, :, h0 * P:(h0 + nh) * P].transpose(1, 0, 2))
        for k in range(NWC):
            im[f"w2_{k}"] = np.ascontiguousarray(
                w2_r[k * HC:(k + 1) * HC].transpose(1, 0, 2))
        for b, (n0, cb) in enumerate(starts):
            im[f"x_{b}"] = np.ascontiguousarray(
                x_r[n0:n0 + cb].transpose(2, 1, 0))
        in_maps.append(im)
    res = run_bass_kernel_spmd(nc_exp, in_maps, core_ids=core_ids)

    # ---- Host: un-permute and combine the two expert contributions ----
    out = np.zeros((B, O), dtype=np.float32)
    for e in range(E):
        n_e = len(idx_e[e])
        if n_e:
            out[idx_e[e]] += res.results[e]["y"][:n_e]
    if np.any(b2):
        # b2 is applied host-side: out += sum_k w_k * b2[expert_k]
        out += w_top[:, 0:1] * b2[order[:, 0]] + w_top[:, 1:2] * b2[order[:, 1]]
    return out
